# revision 1
# baseline (speedup 1.0000x reference)
"""CQVAE loss kernel for Trainium2, data-parallel over batch on 8 NeuronCores.

loss = kld(qy) + mse(gather(rzs), zs[:, :Sg]) + bias(best, best_gt)
       + bias(gather(pts), gts)
where bias(p, g) = mse(p, g) + 10 * mse(p[..., MARK, :], g[..., MARK, :]).

Each core handles 16 of the 128 batches: the mapping-gathers run on-device
via indirect DMA (one row per partition) and squared-difference sums are
reduced per partition on the vector/scalar engines.  Each core ships its
[128, 32] per-partition stats tile; the host folds partitions and cores.
"""

import sys

import numpy as np

try:
    import concourse  # noqa: F401
except ImportError:  # pragma: no cover
    sys.path.insert(0, "/opt/trn_rl_repo")

import concourse.bass as bass
import concourse.mybir as mybir
import concourse.tile as tile
from concourse import bacc
from concourse.bass_utils import run_bass_kernel_spmd

F32 = mybir.dt.float32
I32 = mybir.dt.int32
AX = mybir.AxisListType
OP = mybir.AluOpType
ACTF = mybir.ActivationFunctionType

NCORES = 8
B, S, SG, D, P, V = 128, 256, 128, 1024, 118, 64
BL = B // NCORES  # batches per core
P2 = 2 * P  # 236 floats per point-row
MARK = (0, 29, 88, 117)
EPS = 1e-20
ALPHA = 10.0

KB = 8  # gts/pts batches per bias group
BLB = B // NCORES  # best rows per core
NSTAT = 32
AE0 = 16  # stats columns 16.. hold per-piece ae accumulators

_module = None
last_results = None  # BassKernelResults of the most recent run (for profiling)


def _build_module():
    nc = bacc.Bacc()

    zs = nc.dram_tensor("zs", [BL * S, D], F32, kind="ExternalInput")
    rzs = nc.dram_tensor("rzs", [BL * S, D], F32, kind="ExternalInput")
    pts = nc.dram_tensor("pts", [BL * S, P2], F32, kind="ExternalInput")
    gts = nc.dram_tensor("gts", [BL * SG, P2], F32, kind="ExternalInput")
    qy = nc.dram_tensor("qy", [BL * S, V], F32, kind="ExternalInput")
    best = nc.dram_tensor("best", [BLB, P2], F32, kind="ExternalInput")
    best_gt = nc.dram_tensor("best_gt", [BLB, P2], F32, kind="ExternalInput")
    # idx[i, b] = b*S + mapping[b, i]: flat row into the per-core rzs/pts shard
    idx = nc.dram_tensor("idx", [SG, BL], I32, kind="ExternalInput")
    out = nc.dram_tensor("out", [128, NSTAT], F32, kind="ExternalOutput")

    QCOLS = BL * S * V // 128  # 2048
    QN = BL * S // 128  # 32 rows per partition
    KA = 2  # ae batches per group

    with tile.TileContext(nc) as tc:
        with (
            tc.tile_pool(name="ae", bufs=7) as ae,
            tc.tile_pool(name="sm", bufs=2) as sm,
            tc.tile_pool(name="cst", bufs=1) as cst,
        ):
            idx_t = cst.tile([SG, BL], I32)
            nc.sync.dma_start(idx_t[:], idx[:])

            # stats columns: 0=bias_sq 1=bias_mark_sq 2=kld_num 3=best_sq
            #                4=best_mark_sq; 16.. = per-piece ae_sq
            stats = cst.tile([128, NSTAT], F32)
            nc.vector.memset(stats[:], 0.0)
            acc_b = cst.tile([128, KB * P2], F32)
            nc.vector.memset(acc_b[:], 0.0)

            # --- KLD: sum q * (log(q + eps) - log(1/V)) via log(V*q + V*eps) ---
            qy_t = cst.tile([128, QCOLS], F32)
            nc.scalar.dma_start(
                qy_t[:].rearrange("p (n v) -> p n v", v=V),
                qy[:].rearrange("(p n) v -> p n v", n=QN),
            )
            lg = cst.tile([128, QCOLS], F32)
            ebias = cst.tile([128, 1], F32)
            nc.vector.memset(ebias[:], float(V) * EPS)
            nc.scalar.activation(lg[:], qy_t[:], ACTF.Ln, bias=ebias[:], scale=float(V))
            nc.vector.scalar_tensor_tensor(
                out=lg[:],
                in0=lg[:],
                scalar=0.0,
                in1=qy_t[:],
                op0=OP.subtract,
                op1=OP.mult,
                accum_out=stats[:, 2:3],
            )

            # --- BEST: per-core shard [BLB, P2] ---
            bt = sm.tile([BLB, P2], F32, tag="bt")
            nc.scalar.dma_start(bt[:], best[:])
            bgt = sm.tile([BLB, P2], F32, tag="bgt")
            nc.scalar.dma_start(bgt[:], best_gt[:])
            nc.vector.tensor_sub(bt[:], bt[:], bgt[:])
            nc.vector.tensor_mul(bt[:], bt[:], bt[:])
            nc.vector.reduce_sum(out=stats[:BLB, 3:4], in_=bt[:], axis=AX.X)
            bm4 = cst.tile([BLB, 4], F32)
            for j, m in enumerate(MARK):
                nc.vector.reduce_sum(
                    out=bm4[:, j : j + 1], in_=bt[:, 2 * m : 2 * m + 2], axis=AX.X
                )
            nc.vector.reduce_sum(out=stats[:BLB, 4:5], in_=bm4[:], axis=AX.X)

            # --- interleaved AE + BIAS groups ---
            # AE: sum (rzs[b, map[b,i]] - zs[b, i])^2, two batches per group.
            # BIAS: per-column accumulation of (pts_g - gts)^2, KB batches/group.
            zs_r = zs[:].rearrange("(b s) d -> s b d", s=S)
            gts_r = gts[:].rearrange("(b p) c -> p b c", p=SG)
            # AE pieces: 2-batch groups, then two singles for a short tail chain
            ae_pieces = [(g * KA, KA) for g in range(7)] + [(14, 1), (15, 1)]

            def bias_tiles(h):
                b0 = h * KB
                gt8 = sm.tile([128, KB * P2], F32, tag="gt8")
                nc.scalar.dma_start(
                    gt8[:].rearrange("p (k c) -> p k c", c=P2),
                    gts_r[:, b0 : b0 + KB, :],
                )
                pg8 = sm.tile([128, KB * P2], F32, tag="pg8")
                return gt8, pg8

            def pts_gathers(pg8, b0, k0, k1):
                for k in range(k0, k1):
                    nc.gpsimd.indirect_dma_start(
                        out=pg8[:, (k * P2) : ((k + 1) * P2)],
                        out_offset=None,
                        in_=pts[:],
                        in_offset=bass.IndirectOffsetOnAxis(
                            ap=idx_t[:, b0 + k : b0 + k + 1], axis=0
                        ),
                    )

            def bias_compute(gt8, pg8):
                nc.vector.tensor_sub(pg8[:], pg8[:], gt8[:])
                nc.scalar.activation(pg8[:], pg8[:], ACTF.Square)
                nc.vector.tensor_add(acc_b[:], acc_b[:], pg8[:])

            def ae_piece(i):
                b0, ka = ae_pieces[i]
                zt = ae.tile([128, ka * D], F32, tag="zt")
                nc.sync.dma_start(
                    zt[:].rearrange("p (k d) -> p k d", d=D),
                    zs_r[0:SG, b0 : b0 + ka, :],
                )
                rg = ae.tile([128, ka * D], F32, tag="rg")
                for k in range(ka):
                    nc.gpsimd.indirect_dma_start(
                        out=rg[:, (k * D) : ((k + 1) * D)],
                        out_offset=None,
                        in_=rzs[:],
                        in_offset=bass.IndirectOffsetOnAxis(
                            ap=idx_t[:, b0 + k : b0 + k + 1], axis=0
                        ),
                    )
                nc.vector.tensor_sub(rg[:], rg[:], zt[:])
                nc.scalar.activation(
                    rg[:], rg[:], ACTF.Square,
                    accum_out=stats[:, AE0 + i : AE0 + i + 1],
                )

            # lead with big rzs gathers; spread the small pts gathers so
            # Q7 descriptor emission never bunches
            ae_piece(0)
            ae_piece(1)
            gt8_0, pg8_0 = bias_tiles(0)
            pts_gathers(pg8_0, 0, 0, 4)
            ae_piece(2)
            pts_gathers(pg8_0, 0, 4, 8)
            ae_piece(3)
            bias_compute(gt8_0, pg8_0)
            gt8_1, pg8_1 = bias_tiles(1)
            pts_gathers(pg8_1, KB, 0, 4)
            ae_piece(4)
            pts_gathers(pg8_1, KB, 4, 8)
            ae_piece(5)
            bias_compute(gt8_1, pg8_1)
            for i in range(6, len(ae_pieces)):
                ae_piece(i)

            # --- fold bias accumulator into stats ---
            nc.vector.reduce_sum(out=stats[:, 0:1], in_=acc_b[:], axis=AX.X)
            bk4 = cst.tile([128, 4], F32)
            acc_b3 = acc_b[:].rearrange("p (k c) -> p k c", c=P2)
            for j, m in enumerate(MARK):
                nc.vector.reduce_sum(
                    out=bk4[:, j : j + 1],
                    in_=acc_b3[:, :, 2 * m : 2 * m + 2],
                    axis=AX.XY,
                )
            nc.vector.reduce_sum(out=stats[:, 1:2], in_=bk4[:], axis=AX.X)

            # ship per-partition stats; the host folds the 128 partitions
            nc.sync.dma_start(out[:], stats[:])

    nc.compile()
    return nc


def kernel(
    zs, rzs, pts, best, qy, gts, best_gt, mapping, vector_dims, **trace_kwargs
):
    global _module, last_results
    vd = int(np.asarray(vector_dims))
    assert vd == V, f"kernel compiled for vector_dims={V}, got {vd}"

    if _module is None:
        _module = _build_module()

    zs = np.asarray(zs, dtype=np.float32)
    rzs = np.asarray(rzs, dtype=np.float32)
    pts = np.asarray(pts, dtype=np.float32)
    gts = np.asarray(gts, dtype=np.float32)
    qy = np.asarray(qy, dtype=np.float32)
    mapping = np.asarray(mapping).astype(np.int32)
    best2 = np.ascontiguousarray(np.asarray(best, dtype=np.float32).reshape(B, P2))
    bgt2 = np.ascontiguousarray(np.asarray(best_gt, dtype=np.float32).reshape(B, P2))

    base = (np.arange(BL, dtype=np.int32) * S)[:, None]
    in_maps = []
    for c in range(NCORES):
        sl = slice(c * BL, (c + 1) * BL)
        in_maps.append(
            {
                "zs": zs[sl].reshape(BL * S, D),
                "rzs": rzs[sl].reshape(BL * S, D),
                "pts": pts[sl].reshape(BL * S, P2),
                "gts": gts[sl].reshape(BL * SG, P2),
                "qy": qy[sl].reshape(BL * S, V),
                "best": np.ascontiguousarray(best2[sl]),
                "best_gt": np.ascontiguousarray(bgt2[sl]),
                "idx": np.ascontiguousarray((mapping[sl] + base).T),
            }
        )

    last_results = run_bass_kernel_spmd(
        _module, in_maps, list(range(NCORES)), **trace_kwargs
    )
    parts = np.stack(
        [
            np.asarray(r["out"], dtype=np.float64).reshape(128, NSTAT).sum(axis=0)
            for r in last_results.results
        ]
    )
    tot = parts.sum(axis=0)

    ae_loss = tot[AE0:].sum() / (B * SG * D)
    bias_loss = tot[0] / (B * SG * P2) + ALPHA * tot[1] / (B * SG * 2 * len(MARK))
    kld_loss = tot[2] / (B * S)
    best_mse = tot[3] / (B * P2) + ALPHA * tot[4] / (B * 2 * len(MARK))

    return np.array(kld_loss + ae_loss + best_mse + bias_loss, dtype=np.float32)

